# revision 52
# baseline (speedup 1.0000x reference)
"""Multi-head causal attention (B=2, T=2048, E=1024, H=16, D=64) on 8 TRN2 cores.

Sharding: tensor-parallel over heads. Core c owns heads {2c, 2c+1} for both
batches. Each core computes its heads' q/k/v projections, causal attention,
and a partial output projection z_c = out_c @ Wo[:, 128c:128c+128].T.
Host combines: z = sum_c z_c + bo.

v2 design:
- Q/K projections in fp8e4 DoubleRow (contraction 256/matmul), V in bf16.
  Weights prescaled x64 for fp8 range; k eviction rescales by 1/(64*64*8).
- Scores ST[s, t] = q_s . k_t computed per (pair-of-s-chunks, head) with
  pair-shared causal windows; causal mask applied on the PE as an extra
  accumulating matmul adding -30 to dead entries (maskL stationary,
  maskRc moving), so exp feeds A@V directly.
- exp on Act engine only (with EXP_BIAS), output fp8 (bf16 for tb=0).
- A@V in fp8 DoubleRow over s-chunk pairs; tb=0 in bf16 so early tokens
  avoid fp8 value noise. Ones column in V gives the softmax denominator.
- Emission is software-pipelined (AV lags QK by LAG units) and the two
  batches' attention blocks are interleaved to balance PE vs Act.
"""

import numpy as np
import ml_dtypes

import concourse.bacc as bacc
import concourse.mybir as mybir
import concourse.tile as tile
from concourse.bass_utils import run_bass_kernel_spmd


def _make_runner(nc):
    """Persistent jitted SPMD callable (avoids per-call jit re-trace)."""
    import jax
    from jax.sharding import Mesh, NamedSharding, PartitionSpec
    try:
        from jax.experimental.shard_map import shard_map
    except ImportError:
        shard_map = jax.shard_map
    from concourse.bass2jax import (_bass_exec_p, install_neuronx_cc_hook,
                                    partition_id_tensor)

    install_neuronx_cc_hook()
    partition_name = (nc.partition_id_tensor.name
                      if nc.partition_id_tensor else None)
    in_names, out_names, out_avals, zero_outs = [], [], [], []
    for alloc in nc.m.functions[0].allocations:
        if not isinstance(alloc, mybir.MemoryLocationSet):
            continue
        name = alloc.memorylocations[0].name
        if alloc.kind == "ExternalInput":
            if name != partition_name:
                in_names.append(name)
        elif alloc.kind == "ExternalOutput":
            shape = tuple(alloc.tensor_shape)
            dtype = mybir.dt.np(alloc.dtype)
            out_names.append(name)
            out_avals.append(jax.core.ShapedArray(shape, dtype))
            zero_outs.append(np.zeros(shape, dtype))
    all_in = list(in_names) + list(out_names)
    if partition_name is not None:
        all_in.append(partition_name)

    def _body(*args):
        operands = list(args)
        if partition_name is not None:
            operands.append(partition_id_tensor())
        return tuple(_bass_exec_p.bind(
            *operands, out_avals=tuple(out_avals), in_names=tuple(all_in),
            out_names=tuple(out_names), lowering_input_output_aliases=(),
            sim_require_finite=True, sim_require_nnan=True, nc=nc))

    devices = jax.devices()[:N_CORES]
    mesh = Mesh(np.asarray(devices), ("core",))
    spec = NamedSharding(mesh, PartitionSpec("core"))
    rspec = NamedSharding(mesh, PartitionSpec())
    in_specs = tuple(
        (PartitionSpec() if nm in REPLICATED else PartitionSpec("core"))
        for nm in in_names) + (PartitionSpec("core"),) * len(out_names)
    fn = jax.jit(
        shard_map(_body, mesh=mesh, in_specs=in_specs,
                  out_specs=(PartitionSpec("core"),) * len(out_names),
                  check_rep=False),
        keep_unused=True)
    zeros_dev = [
        jax.device_put(np.zeros((N_CORES * z.shape[0], *z.shape[1:]), z.dtype),
                       spec) for z in zero_outs
    ]

    def run(in_maps):
        concat = [
            jax.device_put(np.asarray(in_maps[0][nm]), rspec)
            if nm in REPLICATED else
            jax.device_put(
                np.concatenate([np.asarray(in_maps[c][nm])
                                for c in range(N_CORES)], axis=0), spec)
            for nm in in_names
        ]
        outs = fn(*concat, *zeros_dev)
        fulls = [np.asarray(outs[i]).reshape(N_CORES, *out_avals[i].shape)
                 for i in range(len(out_names))]
        return [{nm: fulls[i][c] for i, nm in enumerate(out_names)}
                for c in range(N_CORES)]

    return run

N_CORES = 8
B, T, E = 2, 2048, 1024
H, D = 16, 64
HPC = H // N_CORES          # heads per core = 2
F = HPC * D                 # local feature cols = 128
NTB = T // 512              # 4 t-blocks
NSC = T // 128              # 16 s-chunks
WS = 64.0                   # fp8 weight prescale
KSCALE = 1.0 / (WS * WS * 8.0)  # folded into k eviction (incl D^-0.5)
EXP_BIAS = -2.0
NEG = -30.0

REPLICATED = {"xb", "x8", "identb", "maskb", "mask8"}

F32 = mybir.dt.float32
F16 = mybir.dt.float16
F32R = mybir.dt.float32r
BF16 = mybir.dt.bfloat16
FP8 = mybir.dt.float8e4
EXP = mybir.ActivationFunctionType.Exp
DR = mybir.MatmulPerfMode.DoubleRow


def build_nc(rep=1, cfg=None):
    cfg = dict(cfg or {})
    lag = cfg.get("lag", 2)
    nc = bacc.Bacc("TRN2", target_bir_lowering=False, debug=False,
                   num_devices=N_CORES)

    xb = nc.dram_tensor("xb", [B, E, T], BF16, kind="ExternalInput").ap()
    x8 = nc.dram_tensor("x8", [B, E, T], FP8, kind="ExternalInput").ap()
    wq8 = nc.dram_tensor("wq8", [E, F], FP8, kind="ExternalInput").ap()
    wk8 = nc.dram_tensor("wk8", [E, F], FP8, kind="ExternalInput").ap()
    wv = nc.dram_tensor("wv", [E, F], BF16, kind="ExternalInput").ap()
    wot = nc.dram_tensor("wot", [F, E], BF16, kind="ExternalInput").ap()
    identb = nc.dram_tensor("identb", [128, 128], F32,
                            kind="ExternalInput").ap()
    maskb = nc.dram_tensor("maskb", [128, 128], BF16,
                           kind="ExternalInput").ap()
    mask8 = nc.dram_tensor("mask8", [128, 128], FP8,
                           kind="ExternalInput").ap()
    zp = nc.dram_tensor("zp", [B, T, E], F16, kind="ExternalOutput").ap()

    with tile.TileContext(nc) as tc:
        with (
            tc.tile_pool(name="const", bufs=1) as cpool,
            tc.tile_pool(name="xbp", bufs=32) as xbp,
            tc.tile_pool(name="x8p", bufs=16) as x8p,
            tc.tile_pool(name="proj", bufs=4) as projp,
            tc.tile_pool(name="v2p", bufs=32) as v2p,
            tc.tile_pool(name="v2bp", bufs=32) as v2bp,
            tc.tile_pool(name="ptp", bufs=4) as ptp,
            tc.tile_pool(name="smallp", bufs=4) as smallp,
            tc.tile_pool(name="zsbp", bufs=3) as zsbp,
            tc.tile_pool(name="ps_s", bufs=2, space="PSUM") as ps_s,
            tc.tile_pool(name="ps_o", bufs=2, space="PSUM") as ps_o,
            tc.tile_pool(name="ps_t", bufs=2, space="PSUM") as ps_t,
        ):
            # ---- constants (loaded once) ----
            ident = cpool.tile([128, 128], F32, tag="ident")
            nc.scalar.dma_start(ident[:], identb)
            mskb = cpool.tile([128, 128], BF16, tag="mskb")
            nc.scalar.dma_start(mskb[:], maskb)
            msk8 = cpool.tile([128, 128], FP8, tag="msk8")
            nc.scalar.dma_start(msk8[:], mask8)
            ebias = cpool.tile([128, 1], F32, tag="ebias")
            nc.vector.memset(ebias[:], EXP_BIAS)
            ones64 = cpool.tile([1, 64], BF16, tag="ones64")
            nc.vector.memset(ones64[:], 1.0)
            wq_sb = cpool.tile([128, 2 * E // 2], FP8, tag="wq")  # [128,(c j f)]
            wk_sb = cpool.tile([128, 2 * E // 2], FP8, tag="wk")
            for t_, src in ((wq_sb, wq8), (wk_sb, wk8)):
                nc.scalar.dma_start(
                    t_.rearrange("p (c j f) -> p c j f", c=4, j=2),
                    src.rearrange("(c j p) f -> p c j f", j=2, p=128))
            wv_sb = cpool.tile([128, E], BF16, tag="wv")  # [128, (c f)]
            nc.scalar.dma_start(
                wv_sb.rearrange("p (c f) -> p c f", c=8),
                wv.rearrange("(c p) f -> p c f", p=128))
            wot_sb = cpool.tile([F, E], BF16, tag="wot")
            nc.scalar.dma_start(wot_sb[:], wot)
            wqr = wq_sb.rearrange("p (c j f) -> p c j f", c=4, j=2)
            wkr = wk_sb.rearrange("p (c j f) -> p c j f", c=4, j=2)
            wvr = wv_sb.rearrange("p (c f) -> p c f", c=8)

            def body():
                st = {}

                def emit_loads(b):
                    xbt = [[None] * 2 for _ in range(8)]   # [ec][half]
                    x8t = [[None] * 2 for _ in range(4)]   # [c][half]
                    for half in range(2):
                        for ec in range(8):
                            t_ = xbp.tile([128, 1024], BF16, tag="xb")
                            nc.sync.dma_start(
                                t_[:], xb[b, ec * 128:(ec + 1) * 128,
                                          half * 1024:(half + 1) * 1024])
                            xbt[ec][half] = t_
                        for c in range(4):
                            t_ = x8p.tile([128, 2, 1024], FP8, tag="x8")
                            nc.scalar.dma_start(
                                t_[:],
                                x8[b, c * 256:(c + 1) * 256,
                                   half * 1024:(half + 1) * 1024]
                                .rearrange("(j p) t -> p j t", j=2))
                            x8t[c][half] = t_
                    st[b] = {"xbt": xbt, "x8t": x8t}

                def proj_steps(b):
                    """Allocate proj tiles now; return per-(half,proj) emit
                    steps."""
                    xbt, x8t = st[b]["xbt"], st[b]["x8t"]
                    qT2, kT2, vT2 = [], [], []
                    for half in range(2):
                        qT2.append(projp.tile([128, T // 2], BF16, tag="qT2",
                                              name=f"q{b}{half}"))
                        kT2.append(projp.tile([128, T // 2], BF16, tag="kT2",
                                              name=f"k{b}{half}"))
                        vT2.append(projp.tile([128, T // 2], F32, tag="vT2",
                                              name=f"v{b}{half}"))
                    st[b].update(qT2=qT2, kT2=kT2, vT2=vT2)

                    def mk(half, nm):
                        def step():
                            ps = ps_s.tile([128, 1024], F32, tag="sp",
                                           name=f"ps{b}{half}{nm}")
                            for sub in range(2):
                                sl = ps[:, sub * 512:(sub + 1) * 512]
                                if nm == "v":
                                    for ec in range(8):
                                        nc.tensor.matmul(
                                            sl, wvr[:, ec],
                                            xbt[ec][half][:, sub * 512:
                                                          (sub + 1) * 512],
                                            start=(ec == 0), stop=(ec == 7))
                                else:
                                    w = wqr if nm == "q" else wkr
                                    for c in range(4):
                                        nc.tensor.matmul(
                                            sl, w[:, c],
                                            x8t[c][half][:, :,
                                                         sub * 512:
                                                         (sub + 1) * 512],
                                            start=(c == 0), stop=(c == 3),
                                            perf_mode=DR)
                            if nm == "q":
                                nc.vector.tensor_copy(qT2[half][:], ps[:])
                            elif nm == "k":
                                nc.vector.tensor_scalar_mul(
                                    kT2[half][:], ps[:], KSCALE)
                            else:
                                nc.vector.tensor_copy(vT2[half][:], ps[:])
                        return step

                    return [mk(half, nm) for half in range(2)
                            for nm in ("q", "k", "v")]

                def v2_steps(b):
                    """Per-pair transpose+pack steps; v2b list fills in as
                    steps run."""
                    v2b = []
                    st[b]["v2b"] = v2b

                    def mk(pp):
                        def step():
                            vT2 = st[b]["vT2"]
                            tpw = ps_t.tile([128, 512], F32, tag="tp",
                                            name=f"tp{b}{pp}")
                            for j in range(2):
                                si = 2 * pp + j
                                nc.tensor.matmul(
                                    tpw[:, j * 128:(j + 1) * 128],
                                    vT2[si // 8][:, (si % 8) * 128:
                                                 (si % 8 + 1) * 128],
                                    ident[:], is_transpose=True)
                            tpr = tpw[:, 0:256].rearrange(
                                "p (j g d) -> p j g d", j=2, g=2)
                            for j in range(2):
                                tb16 = v2bp.tile([128, 130], BF16, tag="v2b")
                                rb = tb16.rearrange("p (g d) -> p g d", g=2)
                                nc.vector.memset(rb[:, :, 64:65], 1.0)
                                nc.vector.tensor_copy(rb[:, :, 0:64],
                                                      tpr[:, j])
                                v2b.append(tb16)
                        return step

                    return [mk(pp) for pp in range(8)]

                # ---- attention as a flat pipelined unit stream ----
                def attn_units(b, tb):
                    """Yield emit-closures; caller pipelines QK vs AV."""
                    qT2, kT2 = st[b]["qT2"], st[b]["kT2"]
                    v2b = st[b]["v2b"]
                    npairs = 2 * tb + 2
                    po = {}

                    def qk(p, h):
                        c0 = 256 if (p == 2 * tb + 1) else 0
                        ps = ps_s.tile([128, 1024], F32, tag="sp")
                        for dp in range(2):
                            si = 2 * p + dp
                            nc.tensor.matmul(
                                ps[:, dp * 512 + c0:(dp + 1) * 512],
                                qT2[si // 8][64 * h:64 * h + 64,
                                             (si % 8) * 128:
                                             (si % 8 + 1) * 128],
                                kT2[tb // 2][64 * h:64 * h + 64,
                                             (tb % 2) * 512 + c0:
                                             (tb % 2 + 1) * 512],
                                start=True, stop=True,
                                skip_group_check=True)
                        # exp -> pt
                        pt = ptp.tile([128, 1024], BF16, tag="ptb")
                        msk = mskb
                        if c0 == 0:
                            nc.scalar.activation(pt[:], ps[:], EXP,
                                                 bias=ebias[:])
                        else:
                            pr = pt.rearrange("p (j t) -> p j t", j=2)
                            sr = ps.rearrange("p (j t) -> p j t", j=2)
                            nc.scalar.activation(pr[:, :, c0:512],
                                                 sr[:, :, c0:512], EXP,
                                                 bias=ebias[:])
                        # causal mask on the Pool engine: zero the
                        # fully-dead 128-block (odd r) and apply the
                        # triangular mask on the diagonal 128-block
                        for dp in range(2):
                            si = 2 * p + dp
                            r = si - 4 * tb
                            if 0 <= r < 4:
                                if r in (1, 3) and r * 128 > c0:
                                    nc.vector.memset(
                                        pt[:, dp * 512 + (r - 1) * 128:
                                           dp * 512 + r * 128], 0.0)
                                dsl = pt[:, dp * 512 + r * 128:
                                         dp * 512 + (r + 1) * 128]
                                nc.vector.tensor_mul(dsl, dsl, msk[:])
                        return pt

                    def av(p, h, pt):
                        if h not in po:
                            po[h] = ps_o.tile([65, 512], F32, tag="op",
                                              name=f"po{h}")
                        for dp in range(2):
                            si = 2 * p + dp
                            r = si - 4 * tb
                            cs = max(r, 0) * 128
                            nc.tensor.matmul(
                                po[h][:, cs:512],
                                v2b[si][:, h * 65:(h + 1) * 65],
                                pt[:, dp * 512 + cs:(dp + 1) * 512],
                                start=(si == 0), stop=(si == 4 * tb + 3),
                                skip_group_check=True)

                    units = [(p, h) for p in range(npairs) for h in range(2)]
                    for u in units:
                        yield ("qk", b, tb, u, qk, av, po)

                def finish_steps(b, tb, po):
                    if cfg.get("skip_finish", False):
                        return []
                    state = {}

                    def norm(h):
                        def step():
                            if "outT" not in state:
                                state["outT"] = smallp.tile(
                                    [128, 512], BF16, tag="outT",
                                    name=f"outT{b}{tb}")
                            outT = state["outT"]
                            rrow = smallp.tile([1, 512], BF16, tag="rrow",
                                               name=f"rr{b}{tb}{h}")
                            with nc.allow_low_precision(
                                    reason="bf16 softmax denom reciprocal"):
                                nc.vector.reciprocal(rrow[:],
                                                     po[h][64:65, :])
                            # broadcast across 64 partitions via PE outer
                            # product, then to SBUF
                            rps = ps_t.tile([128, 512], F32, tag="tp",
                                            name=f"rps{b}{tb}{h}")
                            nc.tensor.matmul(rps[0:64, :], ones64[:],
                                             rrow[:], start=True, stop=True)
                            rbc = smallp.tile([64, 512], F32, tag="rbc",
                                              name=f"rb{b}{tb}{h}")
                            nc.vector.tensor_copy(rbc[:], rps[0:64, :])
                            nc.vector.tensor_mul(
                                outT[64 * h:64 * h + 64, :],
                                po[h][0:64, :], rbc[:])
                        return step

                    def zstep(jp2, jj):
                        def step():
                            outT = state["outT"]
                            if jj == 0:
                                state[jp2] = zsbp.tile([128, 2048], F16,
                                                       tag="zsb",
                                                       name=f"zs{b}{tb}{jp2}")
                            zsb = state[jp2]
                            j = 2 * jp2 + jj
                            for eb in range(2):
                                zt = ps_t.tile([128, 512], F32, tag="tp",
                                               name=f"zt{b}{tb}{j}{eb}")
                                nc.tensor.matmul(
                                    zt[:], outT[:, j * 128:(j + 1) * 128],
                                    wot_sb[:, eb * 512:(eb + 1) * 512],
                                    start=True, stop=True)
                                dstsl = zsb[:, jj * 1024 + eb * 512:
                                            jj * 1024 + (eb + 1) * 512]
                                if eb == 1 and jj == 0 and jp2 == 0:
                                    nc.scalar.copy(dstsl, zt[:])
                                else:
                                    nc.vector.tensor_copy(dstsl, zt[:])
                            if jj == 1:
                                t0r = (4 * tb + 2 * jp2) * 128
                                nc.sync.dma_start(
                                    zp[b, t0r:t0r + 256, :]
                                    .rearrange("(a p) c -> p a c", p=128),
                                    zsb.rearrange("p (a c) -> p a c", a=2))
                        return step

                    return [norm(0), norm(1), zstep(0, 0), zstep(0, 1),
                            zstep(1, 0), zstep(1, 1)]

                # ---- master emission ----
                front_only = cfg.get("front_only", False)
                emit_loads(0)
                p0 = proj_steps(0)
                for s in p0:
                    s()
                v0 = v2_steps(0)
                v0[0]()
                v0[1]()
                emit_loads(1)
                if front_only:
                    for s in proj_steps(1) + v2_steps(1):
                        s()
                    return

                fillers = list(v0[2:]) + proj_steps(1) + v2_steps(1)

                def pump(n):
                    for _ in range(n):
                        if fillers:
                            fillers.pop(0)()

                # interleave order of attention blocks
                order = [(0, 0), (0, 1), (1, 0), (0, 2), (1, 1), (0, 3),
                         (1, 2), (1, 3)]

                def flush_one():
                    bb, tt, u, av, pt, po, last = pending.pop(0)
                    av(*u, pt)
                    if last:
                        fillers.extend(finish_steps(bb, tt, po))

                # flat pipeline across all blocks
                pending = []        # (b, tb, u, av, pt, po, is_last)
                for (b, tb) in order:
                    gen = list(attn_units(b, tb))
                    n = len(gen)
                    for i, (_, bb, tt, u, qk, av, po) in enumerate(gen):
                        pt = qk(*u)
                        pending.append((bb, tt, u, av, pt, po, i == n - 1))
                        if len(pending) > lag:
                            flush_one()
                        pump(2)
                while pending:
                    flush_one()
                    pump(2)
                pump(len(fillers))

            if rep == 1:
                body()
            else:
                with tc.For_i(0, rep, 1, staggered_reset=True):
                    body()

    nc.compile()
    return nc


def make_in_maps(inputs, Wk, Wq, Wv, Wo):
    """Shard full inputs into per-core input maps."""
    xt = np.ascontiguousarray(inputs.transpose(0, 2, 1))
    xbn = xt.astype(ml_dtypes.bfloat16)
    x8n = xt.astype(ml_dtypes.float8_e4m3)
    identb = np.eye(128, dtype=np.float32)
    tri = (np.arange(128)[None, :] >= np.arange(128)[:, None])
    maskb = tri.astype(ml_dtypes.bfloat16)
    mask8 = tri.astype(ml_dtypes.float8_e4m3)
    in_maps = []
    for c in range(N_CORES):
        h0 = HPC * c
        wq2 = np.concatenate([Wq[h0 + i] for i in range(HPC)], axis=1)
        wk2 = np.concatenate([Wk[h0 + i] for i in range(HPC)], axis=1)
        wv2 = np.concatenate([Wv[h0 + i] for i in range(HPC)], axis=1)
        wot = np.ascontiguousarray(Wo[:, F * c:F * (c + 1)].T)
        in_maps.append({
            "xb": xbn,
            "x8": x8n,
            "wq8": (wq2 * WS).astype(ml_dtypes.float8_e4m3),
            "wk8": (wk2 * WS).astype(ml_dtypes.float8_e4m3),
            "wv": wv2.astype(ml_dtypes.bfloat16),
            "wot": wot.astype(ml_dtypes.bfloat16),
            "identb": identb,
            "maskb": maskb,
            "mask8": mask8,
        })
    return in_maps


_NC = None
_RUN = None
DEFAULT_CFG = {}


def kernel(inputs, Wk, Wq, Wv, Wo, bo):
    global _NC, _RUN
    if _NC is None:
        _NC = build_nc(cfg=DEFAULT_CFG)
    in_maps = make_in_maps(inputs, Wk, Wq, Wv, Wo)
    try:
        if _RUN is None:
            _RUN = _make_runner(_NC)
        results = _RUN(in_maps)
    except Exception:
        _RUN = False if _RUN is None else _RUN
        res = run_bass_kernel_spmd(_NC, in_maps,
                                   core_ids=list(range(N_CORES)))
        results = res.results
    z = np.zeros((B, T, E), dtype=np.float32)
    for c in range(N_CORES):
        z += results[c]["zp"].astype(np.float32)
    return z + bo.astype(np.float32)
